# revision 1
# baseline (speedup 1.0000x reference)
"""CLPL loss kernel for Trainium2 (Bass/Tile), data-parallel over 8 NeuronCores.

Reference math per row r (logits L[r, :C], bool candidate mask M[r, :C]):
    cnt     = sum(M)
    empty   = cnt == 0            (empty candidate list -> all classes candidates)
    m       = empty ? all-ones : M
    pos     = sum(L where m) / (empty ? C : cnt)
    neg_cnt = C - (empty ? C : cnt)
    neg     = neg_cnt > 0 ? sum(softplus(L) where !m) / max(neg_cnt, 1) : 0
    loss_r  = softplus(-pos) + neg
    out     = mean_r loss_r

Kernel strategy (per core: 512 rows x 32000 cols, ~82 MB of input, memory regime):
  For each [128, F] tile, four row-stats are produced with fused/accumulated ops
  so each engine streams each element at most ~2x:
    - DVE  scalar_tensor_tensor: ln = (M == 0) * L, accum -> sum of non-candidate
      logits (s_notm).  s_masked = s_all - s_notm.
    - ACT  Softplus(ln) with accum -> neg_sum + softplus(0)*cnt (masked entries
      were zeroed, each contributing exactly ACT-softplus(0); corrected in the
      finalize step using softplus(0) measured on-device from the same table).
    - s_all (row-sum of L) and cnt (row-sum of M): one DVE tensor_reduce or one
      ACT Identity-activation-with-accum per chunk, statically interleaved to
      balance the two engines.
  Per-row finalize is a handful of [128,1] ops; per-sample losses are DMA'd out
  and averaged on the host (the all-reduce of the sharding hint collapses to an
  8-way host gather of 512 floats per core).
"""

import numpy as np

B, C = 4096, 32000
N_CORES = 8
RPC = B // N_CORES  # rows per core = 512
P = 128             # SBUF partitions
F = 4000            # column chunk
N_CH = C // F       # 8 chunks per row


def _build_nc(rows=RPC, cols=C, f=F, native_softplus=False):
    # softplus(x) = Ln(Exp(x) + 1): the neuronxcc act tables on this build
    # have no softplus function (softplus_and_others ironically lacks it),
    # so both HW and CoreSim use Exp followed by Ln with bias=1 — one table
    # load, natural_log_exp_and_others has both.
    import bass_rust as _bass_rust
    import concourse.bacc as bacc
    import concourse.tile as tile
    from concourse import mybir
    from concourse.hw_specs import get_activation_tables

    class _BaccOneActSet(bacc.Bacc):
        """Bacc whose act-table placement is pinned to the single set that
        covers every function this kernel uses. The stock greedy pass picks
        the first set containing each function (exp_and_others for Exp,
        natural_log for Ln), which reloads the ACT tables on every exp<->ln
        transition — 60 loads x 1.3us measured. Emptying every other set
        (positions preserved, so act_func_set_id stays a valid index into
        act_info.json) forces one load of natural_log_exp_and_others."""

        _ACT_SET = "natural_log_exp_and_others"

        def insert_act_table_loads(self):
            has_activation = any(
                isinstance(i, mybir.InstActivation)
                for b in self.main_func.blocks
                for i in b.instructions
            )
            if not has_activation:
                return
            tables = [
                (name, (s if name == self._ACT_SET else set()))
                for name, s in get_activation_tables(self.m.arch).items()
            ]
            _bass_rust.insert_act_table_loads(self, tables)

    fp32 = mybir.dt.float32
    bf16 = mybir.dt.bfloat16
    u8 = mybir.dt.uint8
    AF = mybir.ActivationFunctionType
    OP = mybir.AluOpType
    AX = mybir.AxisListType

    n_ch = cols // f
    n_rt = rows // P
    assert cols % f == 0 and rows % P == 0

    nc = _BaccOneActSet(
        "TRN2", target_bir_lowering=False, debug=False, num_devices=N_CORES
    )
    lg = nc.dram_tensor("logits", [rows, cols], fp32, kind="ExternalInput").ap()
    mk = nc.dram_tensor("cand_mask", [rows, cols], u8, kind="ExternalInput").ap()
    out = nc.dram_tensor("per_sample", [rows, 1], fp32, kind="ExternalOutput").ap()

    with tile.TileContext(nc) as tc:
        with (
            tc.tile_pool(name="lp", bufs=4) as lp,
            tc.tile_pool(name="mp", bufs=4) as mp,
            tc.tile_pool(name="lnp", bufs=3) as lnp,
            tc.tile_pool(name="spp", bufs=3) as spp,
            tc.tile_pool(name="scrp", bufs=2) as scrp,
            tc.tile_pool(name="accp", bufs=2) as accp,
            tc.tile_pool(name="finp", bufs=2) as finp,
            tc.tile_pool(name="constp", bufs=1) as constp,
        ):
            def softplus(out, in_, scale=1.0, accum_out=None, scratch_pool=None):
                if native_softplus:
                    nc.scalar.activation(
                        out=out, in_=in_, func=AF.Softplus, scale=scale,
                        accum_out=accum_out,
                    )
                else:
                    # exp scratch in bf16 for the big chunk tiles (ln reads it
                    # back; 1+e is formed in fp32 inside the ACT pipe, and
                    # exp(0)=1 stays exact so the softplus(0)*cnt correction
                    # is unaffected); fp32 for the [P,1] finalize values.
                    big = in_.free_size() > 1
                    e_t = (scratch_pool or finp).tile(
                        list(in_.shape),
                        bf16 if big else fp32,
                        tag="sp_exp" if big else "sp_exp_fin",
                    )
                    nc.scalar.activation(out=e_t, in_=in_, func=AF.Exp, scale=scale)
                    nc.scalar.activation(
                        out=out, in_=e_t, func=AF.Ln, bias=1.0,
                        accum_out=accum_out,
                    )

            # -softplus(0) exactly as the ACT table computes it
            zt = constp.tile([P, 1], fp32)
            nc.vector.memset(zt, 0.0)
            c0n = constp.tile([P, 1], fp32)
            softplus(c0n, zt)
            nc.vector.tensor_scalar_mul(c0n, c0n, -1.0)

            for rt in range(n_rt):
                r0 = rt * P
                acc_notm = accp.tile([P, n_ch], fp32, tag="acc_notm")
                acc_sp = accp.tile([P, n_ch], fp32, tag="acc_sp")
                acc_m = accp.tile([P, n_ch], fp32, tag="acc_m")
                acc_cnt = accp.tile([P, n_ch], fp32, tag="acc_cnt")

                for j in range(n_ch):
                    cc = j * f
                    # SWDGE dtype-cast during DMA: HBM bytes unchanged, but
                    # every DVE operand becomes 16-bit step-1 -> 2x_1P
                    # eligibility. Accumulators stay fp32 (pre-cast pipeline).
                    Lt = lp.tile([P, f], bf16, tag="Lt")
                    Mt = mp.tile([P, f], bf16, tag="Mt")
                    nc.gpsimd.dma_start(out=Lt, in_=lg[r0 : r0 + P, cc : cc + f])
                    nc.gpsimd.dma_start(out=Mt, in_=mk[r0 : r0 + P, cc : cc + f])

                    # ln = (mask == 0) * logits; accum -> sum of non-candidate L
                    ln_t = lnp.tile([P, f], bf16, tag="ln")
                    nc.vector.scalar_tensor_tensor(
                        out=ln_t,
                        in0=Mt,
                        scalar=0.0,
                        in1=Lt,
                        op0=OP.is_equal,
                        op1=OP.mult,
                        accum_out=acc_notm[:, j : j + 1],
                    )

                    # softplus over ln; masked entries contribute softplus(0).
                    # sp_t is write-only scratch (only accum_out is consumed,
                    # accumulated from the pre-cast fp32 pipeline) -> bf16.
                    sp_t = spp.tile([P, f], bf16, tag="sp")
                    softplus(
                        sp_t, ln_t,
                        accum_out=acc_sp[:, j : j + 1],
                        scratch_pool=spp,
                    )

                    # s_masked = sum(mask * logits) via STT with fused accum.
                    sm_t = scrp.tile([P, f], bf16, tag="sm")
                    nc.vector.scalar_tensor_tensor(
                        out=sm_t, in0=Mt, scalar=1.0, in1=Lt,
                        op0=OP.mult, op1=OP.mult,
                        accum_out=acc_m[:, j : j + 1],
                    )
                    # cnt = sum(mask): (m*1) max m = m. cnt is the only extra
                    # that can run on ACT (Copy w/ accum), so it carries the
                    # DVE<->ACT balance: 2 of 8 chunks on DVE.
                    cn_t = scrp.tile([P, f], bf16, tag="cn")
                    if j in (1, 5):
                        nc.vector.scalar_tensor_tensor(
                            out=cn_t, in0=Mt, scalar=1.0, in1=Mt,
                            op0=OP.mult, op1=OP.max,
                            accum_out=acc_cnt[:, j : j + 1],
                        )
                    else:
                        nc.scalar.activation(
                            out=cn_t, in_=Mt, func=AF.Copy,
                            accum_out=acc_cnt[:, j : j + 1],
                        )

                # ---- finalize this row-tile: all [P, 1] f32 ----
                s_notm = finp.tile([P, 1], fp32, tag="s_notm")
                s_spl = finp.tile([P, 1], fp32, tag="s_spl")
                s_masked = finp.tile([P, 1], fp32, tag="s_masked")
                cnt = finp.tile([P, 1], fp32, tag="cnt")
                nc.vector.tensor_reduce(out=s_notm, in_=acc_notm, axis=AX.X, op=OP.add)
                nc.vector.tensor_reduce(out=s_spl, in_=acc_sp, axis=AX.X, op=OP.add)
                nc.vector.tensor_reduce(out=s_masked, in_=acc_m, axis=AX.X, op=OP.add)
                nc.vector.tensor_reduce(out=cnt, in_=acc_cnt, axis=AX.X, op=OP.add)

                emptyf = finp.tile([P, 1], fp32, tag="emptyf")
                nc.vector.tensor_single_scalar(emptyf, cnt, 0.0, OP.is_equal)

                # s_eff = s_masked + emptyf * s_notm
                # (empty rows: s_masked == 0 and s_notm == sum of all logits)
                s_eff = finp.tile([P, 1], fp32, tag="s_eff")
                nc.vector.scalar_tensor_tensor(
                    out=s_eff, in0=s_notm, scalar=emptyf, in1=s_masked,
                    op0=OP.mult, op1=OP.add,
                )
                # cnt_eff = cnt + emptyf * C
                cnt_eff = finp.tile([P, 1], fp32, tag="cnt_eff")
                nc.vector.scalar_tensor_tensor(
                    out=cnt_eff, in0=emptyf, scalar=float(cols), in1=cnt,
                    op0=OP.mult, op1=OP.add,
                )
                # pos = s_eff / cnt_eff
                rec = finp.tile([P, 1], fp32, tag="rec")
                nc.vector.reciprocal(rec, cnt_eff)
                pos = finp.tile([P, 1], fp32, tag="pos")
                nc.vector.tensor_mul(pos, s_eff, rec)

                # neg_sum = s_spl - softplus(0) * cnt
                neg_sum = finp.tile([P, 1], fp32, tag="neg_sum")
                nc.vector.scalar_tensor_tensor(
                    out=neg_sum, in0=cnt, scalar=c0n, in1=s_spl,
                    op0=OP.mult, op1=OP.add,
                )
                # neg_cnt = C - cnt_eff
                neg_cnt = finp.tile([P, 1], fp32, tag="neg_cnt")
                nc.vector.tensor_scalar(
                    out=neg_cnt, in0=cnt_eff, scalar1=-1.0, scalar2=float(cols),
                    op0=OP.mult, op1=OP.add,
                )
                # neg = (neg_cnt > 0) * (neg_sum / max(neg_cnt, 1))
                neg_den = finp.tile([P, 1], fp32, tag="neg_den")
                nc.vector.tensor_scalar_max(neg_den, neg_cnt, 1.0)
                rec2 = finp.tile([P, 1], fp32, tag="rec2")
                nc.vector.reciprocal(rec2, neg_den)
                nl_raw = finp.tile([P, 1], fp32, tag="nl_raw")
                nc.vector.tensor_mul(nl_raw, neg_sum, rec2)
                neg_loss = finp.tile([P, 1], fp32, tag="neg_loss")
                nc.vector.scalar_tensor_tensor(
                    out=neg_loss, in0=neg_cnt, scalar=0.0, in1=nl_raw,
                    op0=OP.is_gt, op1=OP.mult,
                )

                # per_sample = softplus(-pos) + neg_loss
                pos_sp = finp.tile([P, 1], fp32, tag="pos_sp")
                softplus(pos_sp, pos, scale=-1.0)
                ps = finp.tile([P, 1], fp32, tag="ps")
                nc.vector.tensor_add(ps, pos_sp, neg_loss)

                nc.sync.dma_start(out=out[r0 : r0 + P, :], in_=ps)

    nc.compile()
    return nc


_NC_CACHE = {}


def _get_nc(rows=RPC, cols=C, f=F, native_softplus=False):
    key = (rows, cols, f, native_softplus)
    if key not in _NC_CACHE:
        _NC_CACHE[key] = _build_nc(rows, cols, f, native_softplus)
    return _NC_CACHE[key]


def _make_in_maps(logits, cand_mask):
    lg = np.asarray(logits, dtype=np.float32)
    mk = np.asarray(cand_mask)
    if mk.dtype != np.uint8:
        mk = mk.astype(np.bool_).view(np.uint8)
    in_maps = []
    for c in range(N_CORES):
        sl = slice(c * RPC, (c + 1) * RPC)
        in_maps.append(
            {
                "logits": np.ascontiguousarray(lg[sl]),
                "cand_mask": np.ascontiguousarray(mk[sl]),
            }
        )
    return in_maps


def _run(logits, cand_mask, trace=False, **kw):
    from concourse.bass_utils import run_bass_kernel_spmd

    nc = _get_nc()
    res = run_bass_kernel_spmd(
        nc,
        _make_in_maps(logits, cand_mask),
        core_ids=list(range(N_CORES)),
        trace=trace,
        **kw,
    )
    per_sample = np.concatenate(
        [r["per_sample"].reshape(-1) for r in res.results]
    )
    return np.asarray(per_sample.mean(), dtype=np.float32), res


def kernel(logits, cand_mask):
    out, _ = _run(logits, cand_mask, trace=False)
    return out

